# revision 14
# baseline (speedup 1.0000x reference)
"""Trainium2 Bass kernel for nn_ConstLoss_22746146800082.

loss = mean_{i != j} (Cq[i,j] - Ck[i,j])^2 over the off-diagonal of the
NxN pairwise cosine-similarity matrices of feat_q / feat_k (N=4096, D=1024).

Math note: with q = f / (||f|| + 1e-7), torch-style cosine similarity of q
reduces exactly to (f_i . f_j) / (||f_i|| ||f_j||) (the eps factors cancel,
and the max(.., 1e-8) clamp never binds for this data), so we compute the
cosine matrix of the raw features directly.  The diagonal of Cq - Ck is
O(1e-6) (both diagonals are ~1.0 up to fp rounding), so its contribution to
the sum is O(1e-11) relative - we skip the mask entirely.

Sharding: rows of the similarity matrices are sharded across 8 cores
(512 rows each).  Each core computes its row-block against all 4096
columns, streamed (bf16) from a host-pretransposed copy of the features.
Row norms are computed per-core in fp32 for its own block and AllGathered
(4KB) so every core can scale columns.  Each core reduces its block to one
scalar; the host sums the 8 partials.

Engine split: PE transposes the row-normalized own block (lhsT prep) and
computes the Gram tiles (bf16 in, fp32 PSUM); DVE applies the column
inv-norm scale (PSUM->SBUF); GPSIMD subtracts; ACT squares + row-reduces
into per-(ib,j) accumulators.
"""

import numpy as np

import concourse.bass as bass
import concourse.mybir as mybir
import concourse.tile as tile
from concourse.masks import make_identity
from concourse.vector_clock import ScopedClock
from concourse.bass_utils import run_bass_kernel_spmd

N_CORES = 8
N = 4096
D = 1024
P = 128

B = N // N_CORES          # rows per core (512)
IB = B // P               # 128-row blocks per core (4)
JT = 512                  # column tile
NJ = N // JT              # column tiles (8)
KC = D // P               # contraction chunks (8)

F32 = mybir.dt.float32
BF16 = mybir.dt.bfloat16
ACTF = mybir.ActivationFunctionType


class _TC(tile.TileContext):
    """TileContext whose kernel-tail drain splits its semaphore waits across
    preceding sync-engine NOPs: this container's walrus build rejects a Drain
    carrying more than one sync wait ("Too many sync wait commands")."""

    def _drain_and_barrier(self, tick_clock, wait_clock):
        nc = self.nc
        probe = nc.sync.nop(nofuse=True)
        wait_clock.add_sem_waits(
            probe.ins, ScopedClock({None: tick_clock.global_clock})
        )
        waits = list(probe.ins.sync_info.on_wait or []) if probe.ins.sync_info else []
        if probe.ins.sync_info is not None:
            probe.ins.sync_info.on_wait = waits[:1]
        for w in waits[1:]:
            n2 = nc.sync.nop(nofuse=True)
            n2.ins.sync_info = mybir.SyncInfo(on_wait=[w], on_update=[])
        nc.sync.drain()
        nc.all_engine_barrier()
        popped = nc._tile_sem_poison_stack.pop()
        assert popped is self._sem_poison
        nc.clear_and_free_semaphores(list(self.sems.allocated().values()))
        nc.all_engine_barrier()


MAX_WAITS_PER_INST = 1


def split_excess_waits(nc):
    """walrus (this build) rejects instructions carrying more than a couple
    of semaphore waits.  Hoist excess waits onto injected same-engine NOPs
    placed immediately before the offending instruction."""
    n = 0
    for f in nc.m.functions:
        for bb in f.blocks:
            insts = bb.instructions
            out = []
            changed = False
            for ins in insts:
                si = ins.sync_info
                waits = list(si.on_wait or []) if si is not None else []
                while len(waits) > MAX_WAITS_PER_INST:
                    take = waits[:MAX_WAITS_PER_INST]
                    waits = waits[MAX_WAITS_PER_INST:]
                    nop = mybir.InstNoOp(name=f"I-waitsplit-{n}", ins=[], outs=[])
                    n += 1
                    nop.engine = ins.engine
                    nop.sync_info = mybir.SyncInfo(on_wait=take, on_update=[])
                    out.append(nop)
                    changed = True
                if changed and si is not None:
                    si.on_wait = waits
                out.append(ins)
            if changed:
                bb.instructions = out
    return n


def build_program(sim_mode: bool = False):
    # sim_mode: replace the AllGather with per-core DMA copies so the
    # single-core TimelineSim (which cannot model collectives) can run.
    nc = bass.Bass(
        "TRN2", target_bir_lowering=False, debug=False, num_devices=N_CORES
    )
    fqt = nc.dram_tensor("fqt", [D, N], BF16, kind="ExternalInput").ap()
    fkt = nc.dram_tensor("fkt", [D, N], BF16, kind="ExternalInput").ap()
    fqn = nc.dram_tensor("fqn", [B, D], F32, kind="ExternalInput").ap()
    fkn = nc.dram_tensor("fkn", [B, D], F32, kind="ExternalInput").ap()
    out = nc.dram_tensor("out", [1, 1], F32, kind="ExternalOutput").ap()

    with _TC(nc) as tc:
        with (
            tc.tile_pool(name="consts", bufs=1) as consts,
            tc.tile_pool(name="blocks", bufs=1) as blocks,
            tc.tile_pool(name="bcast", bufs=1) as bcast,
            tc.tile_pool(name="norm_t", bufs=2) as norm_t,
            tc.tile_pool(name="sn_pool", bufs=4) as sn_pool,
            tc.tile_pool(name="rows", bufs=1) as rows,
            tc.tile_pool(name="stream", bufs=2) as stream,
            tc.tile_pool(name="otmp", bufs=4) as otmp,
            tc.tile_pool(name="psum", bufs=6, space="PSUM") as psum,
            tc.tile_pool(name="psum_t", bufs=2, space="PSUM") as psum_t,
            tc.tile_pool(name="dram", bufs=1, space="DRAM") as dram,
        ):
            ones = consts.tile([P, 1], F32)
            nc.vector.memset(ones, 1.0)
            ident16 = consts.tile([P, P], BF16)
            make_identity(nc, ident16)

            # ---- P1: own-block norms + row-normalized lhsT (bf16) ---------
            cc_in = dram.tile([2, B], F32)
            qb16 = blocks.tile([P, KC, B], BF16, name="qb16")
            kb16 = blocks.tile([P, KC, B], BF16, name="kb16")
            for mi, (fn_, b16) in enumerate(((fqn, qb16), (fkn, kb16))):
                n2c = norm_t.tile([P, IB], F32, name="n2c", tag="n2c")
                nats = []
                for ib in range(IB):
                    nat = sn_pool.tile([P, D], F32, name="nat", tag="nat")
                    nc.sync.dma_start(out=nat, in_=fn_[ib * P : (ib + 1) * P, :])
                    tr = norm_t.tile([P, D], F32, name="sqt", tag="sqt")
                    nc.vector.tensor_tensor_reduce(
                        out=tr,
                        in0=nat,
                        in1=nat,
                        scale=1.0,
                        scalar=0.0,
                        op0=mybir.AluOpType.mult,
                        op1=mybir.AluOpType.add,
                        accum_out=n2c[:, ib : ib + 1],
                    )
                    nats.append(nat)
                nn_ = norm_t.tile([P, IB], F32, name="nn", tag="nn")
                nc.scalar.sqrt(nn_, n2c)
                rinvc = norm_t.tile([P, IB], F32, name="rinvc", tag="ri")
                nc.vector.reciprocal(rinvc, nn_)
                # stage to DRAM as a [B] row: element ib*128+p <- rinvc[p, ib]
                dst = bass.AP(
                    cc_in.tensor, cc_in.offset + mi * B, [[1, P], [P, IB]]
                )
                nc.sync.dma_start(out=dst, in_=rinvc)

                # row-normalize (rounding to bf16), transpose through PE
                nbs = []
                for ib in range(IB):
                    nb = sn_pool.tile([P, D], BF16, name="nb", tag="nb")
                    nc.vector.tensor_scalar_mul(
                        nb, nats[ib], rinvc[:, ib : ib + 1]
                    )
                    nbs.append(nb)
                for kc in range(KC):
                    psT = psum_t.tile([P, B], BF16, name="psT", tag="pt")
                    for ib in range(IB):
                        nc.tensor.transpose(
                            psT[:, ib * P : (ib + 1) * P],
                            nbs[ib][:, kc * P : (kc + 1) * P],
                            ident16,
                        )
                    nc.vector.tensor_copy(b16[:, kc, :], psT)

            # ---- P2: AllGather inverse norms (tiny) -----------------------
            cc_out = dram.tile(
                [2 * N_CORES, B], F32,
                addr_space="Local" if sim_mode else "Shared",
            )
            if sim_mode:
                for c in range(N_CORES):
                    nc.sync.dma_start(
                        out=cc_out[2 * c : 2 * c + 2, :], in_=cc_in
                    )
            else:
                nc.gpsimd.collective_compute(
                    "AllGather",
                    mybir.AluOpType.bypass,
                    replica_groups=[list(range(N_CORES))],
                    ins=[cc_in.opt()],
                    outs=[cc_out.opt()],
                )

            # ---- P3: broadcast column inv-norm rows (bf16) ----------------
            # cc_out rows: 2c = q-norms of core c, 2c+1 = k-norms.
            cqb = bcast.tile([P, NJ, JT], F32, name="cqb")
            ckb = bcast.tile([P, NJ, JT], F32, name="ckb")
            q_src = bass.AP(
                cc_out.tensor, cc_out.offset, [[0, P], [2 * B, N_CORES], [1, B]]
            )
            k_src = bass.AP(
                cc_out.tensor, cc_out.offset + B, [[0, P], [2 * B, N_CORES], [1, B]]
            )
            nc.sync.dma_start(out=cqb, in_=q_src)
            nc.sync.dma_start(out=ckb, in_=k_src)

            # ---- P4: main loop over column tiles --------------------------
            acc3 = consts.tile([P, IB, NJ], F32)
            nc.vector.memset(acc3, 0.0)

            for j in range(NJ):
                rq = stream.tile([P, KC, JT], BF16, name="rq", tag="rq")
                rk = stream.tile([P, KC, JT], BF16, name="rk", tag="rk")
                nc.sync.dma_start(
                    out=rq,
                    in_=fqt.rearrange("(kc p) n -> p kc n", p=P)[
                        :, :, j * JT : (j + 1) * JT
                    ],
                )
                nc.sync.dma_start(
                    out=rk,
                    in_=fkt.rearrange("(kc p) n -> p kc n", p=P)[
                        :, :, j * JT : (j + 1) * JT
                    ],
                )
                for ib in range(IB):
                    psq = psum.tile([P, JT], F32, name="psq", tag="ps")
                    psk = psum.tile([P, JT], F32, name="psk", tag="ps")
                    for kc in range(KC):
                        nc.tensor.matmul(
                            psq,
                            lhsT=qb16[:, kc, ib * P : (ib + 1) * P],
                            rhs=rq[:, kc, :],
                            start=(kc == 0),
                            stop=(kc == KC - 1),
                        )
                    for kc in range(KC):
                        nc.tensor.matmul(
                            psk,
                            lhsT=kb16[:, kc, ib * P : (ib + 1) * P],
                            rhs=rk[:, kc, :],
                            start=(kc == 0),
                            stop=(kc == KC - 1),
                        )
                    s1 = otmp.tile([P, JT], F32, name="s1", tag="s1")
                    s2 = otmp.tile([P, JT], F32, name="s2", tag="s2")
                    dd = otmp.tile([P, JT], F32, name="dd", tag="dd")
                    trash = otmp.tile([P, JT], F32, name="trash", tag="tr")
                    nc.vector.tensor_mul(s1, psq, cqb[:, j, :])
                    nc.vector.tensor_mul(s2, psk, ckb[:, j, :])
                    nc.gpsimd.tensor_sub(dd, s1, s2)
                    nc.scalar.activation(
                        out=trash,
                        in_=dd,
                        func=ACTF.Square,
                        accum_out=acc3[:, ib, j : j + 1],
                    )

            # ---- P5: reduce accumulators to one scalar --------------------
            red = rows.tile([P, 1], F32, name="red")
            nc.vector.tensor_reduce(
                red,
                acc3.rearrange("p ib nj -> p (ib nj)"),
                axis=mybir.AxisListType.X,
                op=mybir.AluOpType.add,
            )
            pf = psum_t.tile([1, 1], F32, name="pf", tag="pt")
            nc.tensor.matmul(pf, lhsT=ones, rhs=red, start=True, stop=True)
            s = rows.tile([1, 1], F32, name="s")
            nc.vector.tensor_copy(s, pf)
            nc.sync.dma_start(out=out, in_=s)

    split_excess_waits(nc)
    return nc


_CACHE = {}


def kernel(feat_q: np.ndarray, feat_k: np.ndarray) -> np.ndarray:
    import ml_dtypes

    fq = np.ascontiguousarray(np.asarray(feat_q, dtype=np.float32))
    fk = np.ascontiguousarray(np.asarray(feat_k, dtype=np.float32))
    assert fq.shape == (N, D) and fk.shape == (N, D)

    if "nc" not in _CACHE:
        _CACHE["nc"] = build_program()
    nc = _CACHE["nc"]

    fqt16 = np.ascontiguousarray(fq.T).astype(ml_dtypes.bfloat16)
    fkt16 = np.ascontiguousarray(fk.T).astype(ml_dtypes.bfloat16)
    in_maps = []
    for c in range(N_CORES):
        sl = slice(c * B, (c + 1) * B)
        in_maps.append(
            {
                "fqt": fqt16,
                "fkt": fkt16,
                "fqn": np.ascontiguousarray(fq[sl, :]),
                "fkn": np.ascontiguousarray(fk[sl, :]),
            }
        )
    res = run_bass_kernel_spmd(nc, in_maps, list(range(N_CORES)))
    total = np.float32(0.0)
    for c in range(N_CORES):
        total += res.results[c]["out"][0, 0]
    loss = np.float32(total / np.float32(N * (N - 1)))
    return np.asarray(loss, dtype=np.float32)


if __name__ == "__main__":
    rng = np.random.default_rng(0)
    q = rng.standard_normal((N, D)).astype(np.float32)
    k = rng.standard_normal((N, D)).astype(np.float32)
    print("loss:", kernel(q, k))


# revision 15
# speedup vs baseline: 25526.6672x; 25526.6672x over previous
"""Trainium2 Bass kernel for nn_ConstLoss_22746146800082 (factorized).

loss = mean_{i != j} (Cq[i,j] - Ck[i,j])^2 with Cx the pairwise cosine
matrix of feat_x (N=4096, D=1024).  The eps terms in the reference cancel,
so Cx is the cosine matrix of the raw rows, and the diagonal of Cq - Ck is
~0, so the mask reduces to a constant denominator.

Factorization: ||Cq - Ck||_F^2 = ||Aqq||^2 + ||Akk||^2 - 2 ||Aqk||^2 with
feature-space Grams Aqq = Q^T Q, Akk = K^T K, Aqk = K^T Q of the
row-normalized features (1024x1024 each) - 2.7x fewer MACs than forming
the 4096x4096 similarity matrices.  Row normalization folds into the
stationary operand only (Aqq = (rq^2 . Q)^T Q etc.), so the streamed
moving operand stays raw bf16.

Sharding: output features are sharded across 8 cores (128 each); every
core streams all N samples (natural layout, bf16) and contracts them into
its [128, 1024] slice of all three Grams, held in 6 PSUM banks across the
whole contraction.  Row norms: each core computes fp32 norms of its own
512 rows from the same bf16 data (this cancels the radial part of the
bf16 input rounding; measured 2e-8 end-to-end) and AllGathers the 4KB of
inverse norms.  Each core reduces its Gram slices to one scalar; the host
sums the 8 partials.
"""

import numpy as np

import concourse.bass as bass
import concourse.mybir as mybir
import concourse.tile as tile
from concourse.vector_clock import ScopedClock
from concourse.bass_utils import run_bass_kernel_spmd

N_CORES = 8
N = 4096
D = 1024
P = 128

B = N // N_CORES          # own rows per core (512)
NC = N // P               # sample chunks (32)
MG = 4                    # chunks merged per DMA
NM = NC // MG             # merged groups (8)

F32 = mybir.dt.float32
BF16 = mybir.dt.bfloat16
ACTF = mybir.ActivationFunctionType


class _TC(tile.TileContext):
    """TileContext whose kernel-tail drain splits its semaphore waits across
    preceding sync-engine NOPs: this container's walrus build rejects a Drain
    carrying more than one sync wait ("Too many sync wait commands")."""

    def _drain_and_barrier(self, tick_clock, wait_clock):
        nc = self.nc
        probe = nc.sync.nop(nofuse=True)
        wait_clock.add_sem_waits(
            probe.ins, ScopedClock({None: tick_clock.global_clock})
        )
        waits = list(probe.ins.sync_info.on_wait or []) if probe.ins.sync_info else []
        if probe.ins.sync_info is not None:
            probe.ins.sync_info.on_wait = waits[:1]
        for w in waits[1:]:
            n2 = nc.sync.nop(nofuse=True)
            n2.ins.sync_info = mybir.SyncInfo(on_wait=[w], on_update=[])
        nc.sync.drain()
        nc.all_engine_barrier()
        popped = nc._tile_sem_poison_stack.pop()
        assert popped is self._sem_poison
        nc.clear_and_free_semaphores(list(self.sems.allocated().values()))
        nc.all_engine_barrier()


MAX_WAITS_PER_INST = 2


def split_excess_waits(nc):
    """walrus (this build) rejects instructions carrying more than a couple
    of semaphore waits.  Hoist excess waits onto injected same-engine NOPs
    placed immediately before the offending instruction."""
    n = 0
    for f in nc.m.functions:
        for bb in f.blocks:
            insts = bb.instructions
            out = []
            changed = False
            for ins in insts:
                si = ins.sync_info
                waits = list(si.on_wait or []) if si is not None else []
                while len(waits) > MAX_WAITS_PER_INST:
                    take = waits[:MAX_WAITS_PER_INST]
                    waits = waits[MAX_WAITS_PER_INST:]
                    nop = mybir.InstNoOp(name=f"I-waitsplit-{n}", ins=[], outs=[])
                    n += 1
                    nop.engine = ins.engine
                    nop.sync_info = mybir.SyncInfo(on_wait=take, on_update=[])
                    out.append(nop)
                    changed = True
                if changed and si is not None:
                    si.on_wait = waits
                out.append(ins)
            if changed:
                bb.instructions = out
    return n


def build_program(sim_mode: bool = False):
    nc = bass.Bass(
        "TRN2", target_bir_lowering=False, debug=False, num_devices=N_CORES
    )
    fq16 = nc.dram_tensor("fq16", [N, D], BF16, kind="ExternalInput").ap()
    fk16 = nc.dram_tensor("fk16", [N, D], BF16, kind="ExternalInput").ap()
    # per-core feature-block column slices (raw bf16)
    fqa = nc.dram_tensor("fqa", [N, P], BF16, kind="ExternalInput").ap()
    fka = nc.dram_tensor("fka", [N, P], BF16, kind="ExternalInput").ap()
    # per-core own 512 rows (same bf16 values as the stream)
    fqn = nc.dram_tensor("fqn", [B, D], BF16, kind="ExternalInput").ap()
    fkn = nc.dram_tensor("fkn", [B, D], BF16, kind="ExternalInput").ap()
    out = nc.dram_tensor("out", [1, 1], F32, kind="ExternalOutput").ap()

    with _TC(nc) as tc:
        with (
            tc.tile_pool(name="consts", bufs=1) as consts,
            tc.tile_pool(name="norms", bufs=1) as norms,
            tc.tile_pool(name="ntmp", bufs=2) as ntmp,
            tc.tile_pool(name="stream", bufs=3) as stream,
            tc.tile_pool(name="ablk", bufs=3) as ablk,
            tc.tile_pool(name="ltile", bufs=4) as ltile,
            tc.tile_pool(name="rows", bufs=1) as rows,
            tc.tile_pool(name="psum", bufs=1, space="PSUM") as psum,
            tc.tile_pool(name="psum_f", bufs=1, space="PSUM") as psum_f,
            tc.tile_pool(name="dram", bufs=1, space="DRAM") as dram,
        ):
            ones = consts.tile([P, 1], F32)
            nc.vector.memset(ones, 1.0)

            # ---- own-row norms -> AllGather inverse norms -----------------
            cc_in = dram.tile([2, B], F32)
            for mi, fn_ in enumerate((fqn, fkn)):
                n4 = norms.tile([P, MG, D], BF16, name="n4", tag="n4", bufs=2)
                nc.sync.dma_start(
                    out=n4, in_=fn_.rearrange("(s p) d -> p s d", p=P)
                )
                n2c = ntmp.tile([P, MG], F32, name="n2c", tag="n2c")
                for s in range(MG):
                    tr = ntmp.tile([P, D], F32, name="tr", tag="tr")
                    nc.vector.tensor_mul(tr, n4[:, s, :], n4[:, s, :])
                    nc.vector.tensor_reduce(
                        n2c[:, s : s + 1], tr,
                        axis=mybir.AxisListType.X, op=mybir.AluOpType.add,
                    )
                w = ntmp.tile([P, MG], F32, name="w", tag="w")
                nc.vector.reciprocal(w, n2c)
                rinv = ntmp.tile([P, MG], F32, name="rinv", tag="rv")
                nc.scalar.sqrt(rinv, w)
                dst = bass.AP(
                    cc_in.tensor, cc_in.offset + mi * B, [[1, P], [P, MG]]
                )
                nc.gpsimd.dma_start(out=dst, in_=rinv)

            cc_out = dram.tile(
                [2 * N_CORES, B], F32,
                addr_space="Local" if sim_mode else "Shared",
            )
            if sim_mode:
                for c in range(N_CORES):
                    nc.gpsimd.dma_start(
                        out=cc_out[2 * c : 2 * c + 2, :], in_=cc_in
                    )
            else:
                nc.gpsimd.collective_compute(
                    "AllGather",
                    mybir.AluOpType.bypass,
                    replica_groups=[list(range(N_CORES))],
                    ins=[cc_in.opt()],
                    outs=[cc_out.opt()],
                )

            # all-rows inverse norms as [P, NM, MG]: element (p, g, s) =
            # rinv[global row (g*MG + s)*128 + p]
            # cc_out q rows at element offset 1024*c + (gl%4)*128 + p, where
            # global chunk gl = g*MG+s maps to core c = gl//4, slot gl%4.
            rivq = norms.tile([P, NM, MG], F32, name="rivq")
            rivk = norms.tile([P, NM, MG], F32, name="rivk")
            for t, base in ((rivq, 0), (rivk, B)):
                for g in range(NM):
                    nc.gpsimd.dma_start(
                        out=t[:, g, :],
                        in_=bass.AP(
                            cc_out.tensor,
                            cc_out.offset + base + g * 2 * B,
                            [[1, P], [P, MG]],
                        ),
                    )
            wqq3 = norms.tile([P, NM, MG], F32, name="wqq3")
            wkk3 = norms.tile([P, NM, MG], F32, name="wkk3")
            wqk3 = norms.tile([P, NM, MG], F32, name="wqk3")
            nc.vector.tensor_mul(wqq3, rivq, rivq)
            nc.vector.tensor_mul(wkk3, rivk, rivk)
            nc.vector.tensor_mul(wqk3, rivq, rivk)

            # ---- contraction: 6 PSUM banks across all 32 chunks -----------
            ps = {}
            for g_ in ("qq", "kk", "qk"):
                for h in range(2):
                    ps[(g_, h)] = psum.tile(
                        [P, 512], F32, name=f"ps_{g_}{h}", tag=f"ps_{g_}{h}"
                    )

            for g in range(NM):
                sq4 = stream.tile([P, MG, D], BF16, name="sq4", tag="sq4")
                sk4 = stream.tile([P, MG, D], BF16, name="sk4", tag="sk4")
                nc.sync.dma_start(
                    out=sq4,
                    in_=fq16.rearrange("(g s p) d -> g p s d", s=MG, p=P)[g],
                )
                nc.sync.dma_start(
                    out=sk4,
                    in_=fk16.rearrange("(g s p) d -> g p s d", s=MG, p=P)[g],
                )
                aq4 = ablk.tile([P, MG, P], BF16, name="aq4", tag="aq4")
                ak4 = ablk.tile([P, MG, P], BF16, name="ak4", tag="ak4")
                nc.sync.dma_start(
                    out=aq4,
                    in_=fqa.rearrange("(g s p) a -> g p s a", s=MG, p=P)[g],
                )
                nc.sync.dma_start(
                    out=ak4,
                    in_=fka.rearrange("(g s p) a -> g p s a", s=MG, p=P)[g],
                )
                for s in range(MG):
                    ci = g * MG + s
                    lqq = ltile.tile([P, P], BF16, name="lqq", tag="lqq")
                    lkk = ltile.tile([P, P], BF16, name="lkk", tag="lkk")
                    lqk = ltile.tile([P, P], BF16, name="lqk", tag="lqk")
                    nc.vector.tensor_scalar_mul(
                        lqq, aq4[:, s, :], wqq3[:, g, s : s + 1]
                    )
                    nc.vector.tensor_scalar_mul(
                        lkk, ak4[:, s, :], wkk3[:, g, s : s + 1]
                    )
                    nc.vector.tensor_scalar_mul(
                        lqk, ak4[:, s, :], wqk3[:, g, s : s + 1]
                    )
                    st = dict(start=(ci == 0), stop=(ci == NC - 1))
                    for h in range(2):
                        hs = slice(h * 512, (h + 1) * 512)
                        nc.tensor.matmul(
                            ps[("qq", h)], lhsT=lqq, rhs=sq4[:, s, hs], **st
                        )
                        nc.tensor.matmul(
                            ps[("kk", h)], lhsT=lkk, rhs=sk4[:, s, hs], **st
                        )
                        nc.tensor.matmul(
                            ps[("qk", h)], lhsT=lqk, rhs=sq4[:, s, hs], **st
                        )

            # ---- finish: S = sum(Aqq^2) + sum(Akk^2) - 2 sum(Aqk^2) -------
            accw = consts.tile([P, 6], F32)
            for idx, key in enumerate(ps):
                cp = rows.tile([P, 512], F32, name=f"cp{idx}", tag="cp", bufs=2)
                nc.vector.tensor_copy(cp, ps[key])
                sqv = rows.tile([P, 512], F32, name=f"sqv{idx}", tag="sqv", bufs=2)
                nc.vector.tensor_mul(sqv, cp, cp)
                nc.vector.tensor_reduce(
                    accw[:, idx : idx + 1], sqv,
                    axis=mybir.AxisListType.X, op=mybir.AluOpType.add,
                )
            # red = (qq0+qq1+kk0+kk1) - 2*(qk0+qk1); ps dict order is
            # qq0,qq1,kk0,kk1,qk0,qk1
            r1 = rows.tile([P, 1], F32, name="r1")
            r2 = rows.tile([P, 1], F32, name="r2")
            nc.vector.tensor_reduce(
                r1, accw[:, 0:4], axis=mybir.AxisListType.X, op=mybir.AluOpType.add
            )
            nc.vector.tensor_reduce(
                r2, accw[:, 4:6], axis=mybir.AxisListType.X, op=mybir.AluOpType.add
            )
            red = rows.tile([P, 1], F32, name="red")
            nc.vector.tensor_scalar_mul(red, r2, -2.0)
            nc.vector.tensor_add(red, red, r1)
            pf = psum_f.tile([1, 1], F32, name="pf", tag="pf")
            nc.tensor.matmul(pf, lhsT=ones, rhs=red, start=True, stop=True)
            s_ = rows.tile([1, 1], F32, name="s_")
            nc.vector.tensor_copy(s_, pf)
            nc.sync.dma_start(out=out, in_=s_)

    split_excess_waits(nc)
    return nc


_CACHE = {}


def kernel(feat_q: np.ndarray, feat_k: np.ndarray) -> np.ndarray:
    import ml_dtypes

    fq = np.ascontiguousarray(np.asarray(feat_q, dtype=np.float32))
    fk = np.ascontiguousarray(np.asarray(feat_k, dtype=np.float32))
    assert fq.shape == (N, D) and fk.shape == (N, D)

    if "nc" not in _CACHE:
        _CACHE["nc"] = build_program()
    nc = _CACHE["nc"]

    fq16 = fq.astype(ml_dtypes.bfloat16)
    fk16 = fk.astype(ml_dtypes.bfloat16)
    in_maps = []
    for c in range(N_CORES):
        cs = slice(c * P, (c + 1) * P)
        rs = slice(c * B, (c + 1) * B)
        in_maps.append(
            {
                "fq16": fq16,
                "fk16": fk16,
                "fqa": np.ascontiguousarray(fq16[:, cs]),
                "fka": np.ascontiguousarray(fk16[:, cs]),
                "fqn": np.ascontiguousarray(fq16[rs, :]),
                "fkn": np.ascontiguousarray(fk16[rs, :]),
            }
        )
    res = run_bass_kernel_spmd(nc, in_maps, list(range(N_CORES)))
    total = np.float32(0.0)
    for c in range(N_CORES):
        total += res.results[c]["out"][0, 0]
    loss = np.float32(total / np.float32(N * (N - 1)))
    return np.asarray(loss, dtype=np.float32)


if __name__ == "__main__":
    rng = np.random.default_rng(0)
    q = rng.standard_normal((N, D)).astype(np.float32)
    k = rng.standard_normal((N, D)).astype(np.float32)
    print("loss:", kernel(q, k))
